# revision 36
# baseline (speedup 1.0000x reference)
"""Trainium2 Bass kernel for nn_ConditionPooler (ragged cross-attention pooler).

Algorithm (per core, data-parallel over B=16 scenes, 2 scenes/core on 8 cores):
  scores[n,(h,t)] = feat @ A^T      where A[(h,t),c] = sum_d qh[t,h,d] w_k[h*DH+d,c]
  P = exp(scores/16 + bias)  (A pre-scaled x16 for fp8 range; bias=-30 on padded
                              rows so they vanish from the softmax; b_k_in cancels)
  U[(h,t),c]  = sum_n P[n,(h,t)] feat[n,c]   (PSUM-accumulated across tiles)
  den[1,(h,t)] = sum_n P[n,(h,t)]            (single-row matmul vs ones, PSUM-acc)
  Uhat = U / den; attn_h = Uhat_h @ w_v_h^T; out = attn @ w_o^T + b_o (+ w_o b_v_in)
  h = out + query; z = (h-mu)/std; ff = gelu(z @ (w1*g)^T + b1eff) @ w2^T + b2
  result = h + ff

The scores matmul takes F^T directly from DRAM (host pre-transposes into a
[p, scene, tile, c-chunk, n] layout, so no on-chip transposes or PSUM->SBUF
copies are needed). Everything stays bf16: attention outputs are means over
~2k points, so any per-point weight noise (e.g. fp8 scores ~5%) lands
directly as relative output error — measured 3.4e-2 with fp8e4 DoubleRow
scores vs 5e-3 with bf16. Scenes are padded to a common length P (multiple
of 128); segment boundaries are computed on the host from batch_idx.
"""

import numpy as np

C = 512
T = 32
H = 8
DH = C // H
NCORES = 8

_CACHE = {}


def _apply_tile_patch():
    """This walrus build allows only one sem wait on CTRL-encoded (Drain)
    instructions; TileContext's tail drain carries the whole global clock.
    Split the extra waits onto standalone sync-engine nops."""
    import concourse.tile as tile_mod
    import concourse.mybir as mybir
    from concourse.vector_clock import ScopedClock

    if getattr(tile_mod.TileContext, "_drain_patched", False):
        return

    def _patched(self, tick_clock, wait_clock):
        nc = self.nc
        drain_inst = nc.sync.drain()
        wait_clock.add_sem_waits(
            drain_inst.ins, ScopedClock({None: tick_clock.global_clock})
        )
        si = drain_inst.ins.sync_info
        if si is not None and si.on_wait is not None and len(si.on_wait) > 1:
            waits = list(si.on_wait)
            si.on_wait = waits[:1]
            for w in waits[1:]:
                nop = nc.sync.nop(nofuse=True)
                nsi = nop.ins.sync_info
                if nsi is None:
                    nop.ins.sync_info = mybir.SyncInfo(on_wait=[w], on_update=[])
                else:
                    nsi.on_wait = [w]
        nc.all_engine_barrier()
        assert self.sems is not None
        popped = nc._tile_sem_poison_stack.pop()
        assert popped is self._sem_poison
        nc.clear_and_free_semaphores(list(self.sems.allocated().values()))
        nc.all_engine_barrier()

    tile_mod.TileContext._drain_and_barrier = _patched
    tile_mod.TileContext._drain_patched = True


def _split_multi_waits(nc):
    """This walrus build caps sync waits at 1 per instruction (2 for
    EventSemaphore). Tile emits several on some instructions; hoist the
    extras onto same-engine NoOps inserted just before."""
    import concourse.mybir as mybir

    cnt = [0]
    for f in nc.m.functions:
        for b in f.blocks:
            newlist = []
            for inst in b.instructions:
                si = inst.sync_info
                if si is not None and si.on_wait is not None and len(si.on_wait) > 1:
                    waits = list(si.on_wait)
                    for w in waits[:-1]:
                        cnt[0] += 1
                        nop = mybir.InstNoOp(
                            name=f"I-wsplit-{cnt[0]}", ins=[], outs=[]
                        )
                        nop.engine = inst.engine
                        nop.sync_info = mybir.SyncInfo(on_wait=[w], on_update=[])
                        newlist.append(nop)
                    si.on_wait = waits[-1:]
                newlist.append(inst)
            b.instructions = newlist


def _build(P, S, chunk=6, split=True):
    """Build the per-core SPMD Bass program. P = padded scene length
    (multiple of 128), S = scenes per core."""
    import concourse.bass as bass
    import concourse.mybir as mybir
    import concourse.tile as tile

    _apply_tile_patch()

    f32 = mybir.dt.float32
    bf16 = mybir.dt.bfloat16
    HT = H * T  # 256
    NT = P // 128
    assert P % 128 == 0
    AF = mybir.ActivationFunctionType

    nc = bass.Bass()
    featp = nc.dram_tensor("featp", [S * P, C], bf16, kind="ExternalInput")
    # F^T bf16: [p, s*NT + tile, j(4), n(128)], c = j*128 + p.
    # Per partition one contiguous run per scene.
    featT = nc.dram_tensor("featT", [128, S * NT * 4 * 128], bf16,
                           kind="ExternalInput")
    # A bf16: [p(128), j(4), ht(256)], c = j*128 + p
    a_d = nc.dram_tensor("akT", [C, HT], bf16, kind="ExternalInput")
    biasm_d = nc.dram_tensor("biasm", [128, S * NT], f32, kind="ExternalInput")
    wvT_d = nc.dram_tensor("wvT", [C, C], bf16, kind="ExternalInput")
    woT_d = nc.dram_tensor("woT", [C, C], bf16, kind="ExternalInput")
    w1gT_d = nc.dram_tensor("w1gT", [C, 2 * C], bf16, kind="ExternalInput")
    b1e_d = nc.dram_tensor("b1e", [1, 2 * C], bf16, kind="ExternalInput")
    w2T_d = nc.dram_tensor("w2T", [2 * C, C], bf16, kind="ExternalInput")
    b2e_d = nc.dram_tensor("b2e", [1, C], bf16, kind="ExternalInput")
    qb_d = nc.dram_tensor("qb", [T, C], f32, kind="ExternalInput")
    idb_d = nc.dram_tensor("identb", [128, 128], bf16, kind="ExternalInput")
    outp = nc.dram_tensor("outp", [S * T, C], f32, kind="ExternalOutput")

    NCH = NT // chunk  # chunks per scene
    assert NT % chunk == 0

    with tile.TileContext(nc) as tc:
        with tc.tile_pool(name="const", bufs=1) as const:
            # consts ride the ACT HWDGE ring so they don't delay the feat
            # stream on the sync ring at startup; akT split so scores j=0,1
            # can start before the whole matrix lands
            akT = const.tile([128, 4, HT], bf16, tag="akT")
            av = a_d.rearrange("(j p) f -> p j f", p=128)
            nc.scalar.dma_start(akT[:, :2, :], av[:, :2, :])
            biasm = const.tile([128, S * NT], f32, tag="biasm")
            nc.scalar.dma_start(biasm[:], biasm_d[:])
            nc.scalar.dma_start(akT[:, 2:, :], av[:, 2:, :])
            identb = const.tile([128, 128], bf16, tag="identb")
            nc.scalar.dma_start(identb[:], idb_d[:])
            ones1 = const.tile([1, 2 * T], bf16, tag="ones1")
            nc.vector.memset(ones1[:], 1.0)
            onesc = const.tile([128, 1], bf16, tag="onesc")
            nc.vector.memset(onesc[:], 1.0)
            ones1f = const.tile([1, 1], f32, tag="ones1f")
            nc.vector.memset(ones1f[:], 1.0)

            wvT = const.tile([128, 4, C], bf16, tag="wvT")
            woT = const.tile([128, 4, C], bf16, tag="woT")
            w1gT = const.tile([128, 4, 2 * C], bf16, tag="w1gT")
            w2T = const.tile([128, 8, C], bf16, tag="w2T")
            b1e = const.tile([1, 2 * C], bf16, tag="b1e")
            b2e = const.tile([1, C], bf16, tag="b2e")
            qb2 = const.tile([2 * T, C], f32, tag="qb2")

            def load_epi_weights():
                # SWDGE so these 3.2MB don't head-of-line block the feat
                # stream on HWDGE; issued mid-stream, needed only at the end.
                g = nc.gpsimd
                g.dma_start(wvT[:], wvT_d.rearrange("(j p) f -> p j f", p=128))
                g.dma_start(woT[:], woT_d.rearrange("(j p) f -> p j f", p=128))
                g.dma_start(w1gT[:], w1gT_d.rearrange("(j p) f -> p j f", p=128))
                g.dma_start(w2T[:], w2T_d.rearrange("(j p) f -> p j f", p=128))
                g.dma_start(b1e[:], b1e_d[:])
                g.dma_start(b2e[:], b2e_d[:])
                for s in range(S):
                    g.dma_start(qb2[s * T : (s + 1) * T, :], qb_d[:])

            featv = featp.rearrange(
                "(s g i p) c -> s g p i c", p=128, i=chunk, g=NCH
            )
            # F^T bf16 stream: per (scene, chunk) [128, chunk, 4, 128]
            ftv = featT.rearrange(
                "p (s g i j n) -> s g p i j n", s=S, g=NCH, i=chunk, j=4, n=128
            )

            from collections import deque

            with (
                tc.tile_pool(name="epiA", bufs=1) as epiA,
                tc.tile_pool(name="ftp", bufs=2) as ftp,
                tc.tile_pool(name="fb", bufs=3) as fpool,
                tc.tile_pool(name="sb", bufs=4) as spool,
                tc.tile_pool(name="rd", bufs=2) as rdp,
                tc.tile_pool(name="epi", bufs=1) as epi,
                tc.tile_pool(name="psU", bufs=1, space="PSUM") as psU_pool,
                tc.tile_pool(name="pssc", bufs=2, space="PSUM") as pssc,
                tc.tile_pool(name="psd", bufs=2, space="PSUM") as psd,
                tc.tile_pool(name="pst", bufs=1, space="PSUM") as pst,
                tc.tile_pool(name="psac", bufs=1, space="PSUM") as psac,
            ):
                Uhat = [
                    epiA.tile([128, 2, C], bf16, tag=f"Uh{s}", name=f"Uh{s}")
                    for s in range(S)
                ]
                rdT = epiA.tile([128, 2, S], f32, tag="rdT")
                epsc = epiA.tile([T, 1], f32, tag="epsc")
                nc.vector.memset(epsc[:], 1e-5)

                def emit_U(st):
                    PT, F, sp, ip, Upsp, denp = st
                    for h2 in range(2):
                        nc.tensor.matmul(
                            Upsp[h2][:],
                            PT[:, h2 * 128 : (h2 + 1) * 128],
                            F,
                            start=(ip == 0),
                            stop=(ip == NT - 1),
                        )
                    nc.tensor.matmul(
                        denp[:1, :HT],
                        onesc[:],
                        PT[:],
                        start=(ip == 0),
                        stop=(ip == NT - 1),
                    )

                def finish_scene(st):
                    emit_U(st)
                    sp, Upsp, denp = st[2], st[4], st[5]
                    # rden, transpose rden into [128, 2] (cols 256:258 of
                    # the same den bank), scale U -> Uhat
                    rden = rdp.tile([1, HT], f32, tag="rden")
                    nc.vector.reciprocal(rden[:], denp[:1, :HT])
                    for h2 in range(2):
                        nc.tensor.matmul(
                            denp[:, HT + h2 : HT + h2 + 1],
                            rden[:, h2 * 128 : (h2 + 1) * 128],
                            ones1f[:],
                            start=True,
                            stop=True,
                        )
                    nc.vector.tensor_copy(rdT[:, :, sp], denp[:, HT : HT + 2])
                    # split across ACT and DVE so the halves run in parallel
                    nc.scalar.activation(
                        Uhat[sp][:, 0, :],
                        Upsp[0][:],
                        AF.Copy,
                        scale=rdT[:, 0, sp : sp + 1],
                    )
                    nc.vector.tensor_scalar_mul(
                        Uhat[sp][:, 1, :], Upsp[1][:], rdT[:, 1, sp : sp + 1]
                    )

                def scene_epilogue(sp):
                    """Per-scene tail, as closures spread across the next
                    scene's tile stream (or drained back-to-back at the end)."""
                    UT = epi.tile([128, 4, HT], bf16, tag="UT", name=f"UT{sp}")
                    at_sb = epi.tile(
                        [128, 4, T], bf16, tag="at_sb", name=f"ats{sp}"
                    )
                    h_sb = epi.tile([T, C], f32, tag="h", name=f"h{sp}")
                    mu = epi.tile([T, 1], f32, tag="mu", name=f"mu{sp}")
                    cen = epi.tile([T, C], f32, tag="cen", name=f"cen{sp}")
                    ssq = epi.tile([T, 1], f32, tag="ssq", name=f"ssq{sp}")
                    rstd = epi.tile([T, 1], f32, tag="rstd", name=f"rstd{sp}")
                    z = epi.tile([T, C], bf16, tag="z", name=f"z{sp}")
                    zT = epi.tile([128, 4, T], bf16, tag="zT", name=f"zT{sp}")
                    gm = epi.tile([T, 2, C], bf16, tag="gm", name=f"gm{sp}")
                    gmT = epi.tile([128, 8, T], bf16, tag="gmT", name=f"gmT{sp}")
                    fin = epi.tile([T, C], f32, tag="fin", name=f"fin{sp}")

                    def p_ut():
                        ps_u = pst.tile(
                            [128, 4, HT], bf16, tag="tre", name=f"psu{sp}"
                        )
                        for h2 in range(2):
                            for jc in range(4):
                                nc.tensor.transpose(
                                    ps_u[:, jc, h2 * 128 : (h2 + 1) * 128],
                                    Uhat[sp][:, h2, jc * 128 : (jc + 1) * 128],
                                    identb[:],
                                )
                        nc.vector.tensor_copy(UT[:], ps_u[:])

                    def p_at():
                        # [128, 4, 128] so the per-partition bytes match the
                        # "acc" tag (shares one PSUM bank with ph/pf/po)
                        at_ps = psac.tile(
                            [128, 4, 128], f32, tag="acc", name=f"at{sp}"
                        )
                        for gq in range(4):
                            for hh in range(2):
                                h = 2 * gq + hh
                                for jc in range(4):
                                    nc.tensor.matmul(
                                        at_ps[hh * 64 : (hh + 1) * 64, gq, :T],
                                        wvT[:, jc, h * DH : (h + 1) * DH],
                                        UT[:, jc, h * T : (h + 1) * T],
                                        start=(jc == 0),
                                        stop=(jc == 3),
                                    )
                        nc.any.tensor_copy(at_sb[:], at_ps[:, :, :T])

                    def p_ph():
                        ph = psac.tile([128, C], f32, tag="acc", name=f"ph{sp}")
                        for gq in range(4):
                            nc.tensor.matmul(
                                ph[:T, :],
                                at_sb[:, gq, :],
                                woT[:, gq, :],
                                start=(gq == 0),
                                stop=(gq == 3),
                            )
                        nc.vector.tensor_add(
                            h_sb[:], ph[:T, :], qb2[sp * T : (sp + 1) * T, :]
                        )

                    def p_ln():
                        ssum = epi.tile([T, 1], f32, tag="ssum", name=f"ssum{sp}")
                        nc.vector.reduce_sum(
                            ssum[:], h_sb[:], axis=mybir.AxisListType.X
                        )
                        nc.scalar.mul(mu[:], ssum[:], 1.0 / C)
                        nc.vector.tensor_scalar_sub(cen[:], h_sb[:], mu[:])
                        sq = epi.tile([T, C], f32, tag="sq", name=f"sq{sp}")
                        nc.scalar.activation(
                            sq[:], cen[:], AF.Square, accum_out=ssq[:]
                        )
                        std = epi.tile([T, 1], f32, tag="std", name=f"std{sp}")
                        nc.scalar.activation(
                            std[:], ssq[:], AF.Sqrt, bias=epsc[:], scale=1.0 / C
                        )
                        nc.vector.reciprocal(rstd[:], std[:])
                        nc.vector.tensor_scalar_mul(z[:], cen[:], rstd[:])

                    def p_zt():
                        ps_z = pst.tile(
                            [128, 4, HT], bf16, tag="tre", name=f"psz{sp}"
                        )
                        for jc in range(4):
                            nc.tensor.transpose(
                                ps_z[:, jc, :T],
                                z[:, jc * 128 : (jc + 1) * 128],
                                identb[:T, :T],
                            )
                        nc.any.tensor_copy(zT[:], ps_z[:, :, :T])

                    def p_ff1(half):
                        def run():
                            pf = psac.tile(
                                [128, C], f32, tag="acc", name=f"pf{sp}{half}"
                            )
                            for jc in range(4):
                                nc.tensor.matmul(
                                    pf[:T, :],
                                    zT[:, jc, :],
                                    w1gT[:, jc, half * C : (half + 1) * C],
                                    start=(jc == 0),
                                    stop=False,
                                )
                            nc.tensor.matmul(
                                pf[:T, :],
                                ones1[:, :T],
                                b1e[:, half * C : (half + 1) * C],
                                start=False,
                                stop=True,
                            )
                            nc.scalar.activation(gm[:, half, :], pf[:T, :], AF.Gelu)

                        return run

                    def p_gmt():
                        ps_g = pst.tile(
                            [128, 4, HT], bf16, tag="tre", name=f"psg{sp}"
                        )
                        for half in range(2):
                            for jc in range(4):
                                nc.tensor.transpose(
                                    ps_g[:, jc, half * T : (half + 1) * T],
                                    gm[:, half, jc * 128 : (jc + 1) * 128],
                                    identb[:T, :T],
                                )
                        for half in range(2):
                            nc.vector.tensor_copy(
                                gmT[:, half * 4 : (half + 1) * 4, :],
                                ps_g[:, :, half * T : (half + 1) * T],
                            )

                    def p_ff2():
                        po = psac.tile([128, C], f32, tag="acc", name=f"po{sp}")
                        for k in range(8):
                            nc.tensor.matmul(
                                po[:T, :],
                                gmT[:, k, :],
                                w2T[:, k, :],
                                start=(k == 0),
                                stop=False,
                            )
                        nc.tensor.matmul(
                            po[:T, :], ones1[:, :T], b2e[:], start=False, stop=True
                        )
                        nc.vector.tensor_add(fin[:], h_sb[:], po[:T, :])
                        nc.sync.dma_start(
                            outp[sp * T : (sp + 1) * T, :], fin[:]
                        )

                    return [
                        p_ut,
                        p_at,
                        p_ph,
                        p_ln,
                        p_zt,
                        p_ff1(0),
                        p_ff1(1),
                        p_gmt,
                        p_ff2,
                    ]

                epi_q = deque()

                def load_small_weights():
                    # SWDGE: fires immediately; only the tiny tensors (~70KB)
                    g = nc.gpsimd
                    g.dma_start(b1e[:], b1e_d[:])
                    g.dma_start(b2e[:], b2e_d[:])
                    for s2 in range(S):
                        g.dma_start(qb2[s2 * T : (s2 + 1) * T, :], qb_d[:])

                # Epilogue weights (3.2MB) ride the sync HWDGE queue in
                # pieces so their FIFO position (not issue time) paces them
                # between feat chunks.
                wvv = wvT_d.rearrange("(j p) f -> p j f", p=128)
                wov = woT_d.rearrange("(j p) f -> p j f", p=128)
                w1v = w1gT_d.rearrange("(j p) f -> p j f", p=128)
                w2v = w2T_d.rearrange("(j p) f -> p j f", p=128)
                weight_sched = {
                    (0, min(1, NCH - 1)): [
                        lambda: nc.sync.dma_start(wvT[:], wvv),
                    ],
                    (0, min(2, NCH - 1)): [
                        lambda: nc.sync.dma_start(woT[:], wov),
                    ],
                    (min(1, S - 1), 0): [
                        lambda: nc.sync.dma_start(w1gT[:, :, :C], w1v[:, :, :C]),
                        lambda: nc.sync.dma_start(w1gT[:, :, C:], w1v[:, :, C:]),
                    ],
                    (min(1, S - 1), min(1, NCH - 1)): [
                        lambda: nc.sync.dma_start(w2T[:, :4, :], w2v[:, :4, :]),
                    ],
                    (min(1, S - 1), min(2, NCH - 1)): [
                        lambda: nc.sync.dma_start(w2T[:, 4:, :], w2v[:, 4:, :]),
                    ],
                }

                pend = None
                for s in range(S):
                    Ups = [
                        psU_pool.tile(
                            [128, C], f32, tag=f"U{h2}", name=f"U{s}{h2}"
                        )
                        for h2 in range(2)
                    ]
                    den_ps = psd.tile([128, HT + 2], f32, tag="den", name=f"d{s}")
                    for g in range(NCH):
                        FT = ftp.tile([128, chunk, 4, 128], bf16, tag="FT")
                        Fc = fpool.tile([128, chunk, C], bf16, tag="F")
                        if s == 0 and g == 0:
                            # 2-piece split so the first tiles land early
                            # (HWDGE has ~0.6us flat cost per DMA — don't
                            # over-fragment)
                            cut = min(2, chunk)
                            nc.sync.dma_start(FT[:, :cut], ftv[s, g][:, :cut])
                            nc.sync.dma_start(Fc[:, :cut], featv[s, g][:, :cut])
                            nc.sync.dma_start(FT[:, cut:], ftv[s, g][:, cut:])
                            nc.sync.dma_start(Fc[:, cut:], featv[s, g][:, cut:])
                            load_small_weights()
                        else:
                            nc.sync.dma_start(FT[:], ftv[s, g])
                            nc.sync.dma_start(Fc[:], featv[s, g])
                        for w_fn in weight_sched.get((s, g), ()):
                            w_fn()
                        for ii in range(chunk):
                            i = g * chunk + ii
                            F = Fc[:, ii, :]
                            ps_s = pssc.tile([128, HT], f32, tag="sc")
                            for j in range(4):
                                nc.tensor.matmul(
                                    ps_s[:],
                                    FT[:, ii, j, :],
                                    akT[:, j, :],
                                    start=(j == 0),
                                    stop=(j == 3),
                                )
                            PT = spool.tile([128, HT], bf16, tag="PT")
                            nc.scalar.activation(
                                PT[:],
                                ps_s[:],
                                AF.Exp,
                                bias=biasm[:, s * NT + i : s * NT + i + 1],
                            )
                            if pend is not None:
                                if pend[2] != s:
                                    finish_scene(pend)
                                    epi_q.extend(scene_epilogue(pend[2]))
                                else:
                                    emit_U(pend)
                            if epi_q and i % 2 == 0:
                                # pop every 2nd tile: keeps the overlapped
                                # epilogue behind its weight deliveries
                                epi_q.popleft()()
                            pend = (PT, F, s, i, Ups, den_ps)
                finish_scene(pend)
                epi_q.extend(scene_epilogue(pend[2]))
                while epi_q:
                    epi_q.popleft()()

    if split:
        _split_multi_waits(nc)
    return nc


def _pick_chunk(NT):
    for c in (6, 5, 4, 3, 2, 1):
        if NT % c == 0:
            return c
    return 1


def _host_prep(inputs):
    import ml_dtypes

    bf = ml_dtypes.bfloat16

    feat = np.asarray(inputs["feat"], dtype=np.float32)
    batch_idx = np.asarray(inputs["batch_idx"]).astype(np.int64)
    B = int(np.asarray(inputs["batch_size"]))
    query = np.asarray(inputs["query"], dtype=np.float32)
    g_q = np.asarray(inputs["g_q"], np.float32)
    b_q = np.asarray(inputs["b_q"], np.float32)
    w_q = np.asarray(inputs["w_q"], np.float32)
    w_k = np.asarray(inputs["w_k"], np.float32)
    w_v = np.asarray(inputs["w_v"], np.float32)
    b_q_in = np.asarray(inputs["b_q_in"], np.float32)
    b_v_in = np.asarray(inputs["b_v_in"], np.float32)
    w_o = np.asarray(inputs["w_o"], np.float32)
    b_o = np.asarray(inputs["b_o"], np.float32)
    g_ff = np.asarray(inputs["g_ff"], np.float32)
    b_ff = np.asarray(inputs["b_ff"], np.float32)
    w1 = np.asarray(inputs["w1"], np.float32)
    b1 = np.asarray(inputs["b1"], np.float32)
    w2 = np.asarray(inputs["w2"], np.float32)
    b2 = np.asarray(inputs["b2"], np.float32)

    S = B // NCORES
    counts = np.bincount(batch_idx, minlength=B)
    offs = np.concatenate([[0], np.cumsum(counts)])
    NT = max(1, int(np.ceil(counts.max() / 128)))
    while _pick_chunk(NT) < 3 and NT > 2:
        NT += 1
    P = NT * 128

    featp = np.zeros((NCORES, S * P, C), dtype=bf)
    biasm = np.full((NCORES, S, NT * 128), -30.0, dtype=np.float32)
    for b in range(B):
        c, s = divmod(b, S)
        n = counts[b]
        featp[c, s * P : s * P + n] = feat[offs[b] : offs[b + 1]].astype(bf)
        biasm[c, s, :n] = 0.0
    # bias laid out [128, S*NT] (partition = point-within-tile)
    biasd = (
        biasm.reshape(NCORES, S, NT, 128)
        .transpose(0, 3, 1, 2)
        .reshape(NCORES, 128, S * NT)
    )
    # F^T bf16: [core, p, s, tile, j, n] with c = j*128 + p
    f4 = featp.reshape(NCORES, S, NT, 128, 4, 128)
    # dims: [core, s, tile, n, j, p] -> transpose to [core, p, s, tile, j, n]
    featT = np.ascontiguousarray(f4.transpose(0, 5, 1, 2, 4, 3))

    # query-side fold (host; tiny)
    q = query[0]
    mu = q.mean(-1, keepdims=True)
    var = ((q - mu) ** 2).mean(-1, keepdims=True)
    qn = (q - mu) / np.sqrt(var + 1e-5) * g_q + b_q
    qh = (qn @ w_q.T + b_q_in) / np.sqrt(DH)  # [T, C]
    A = np.einsum(
        "thd,hdc->cht", qh.reshape(T, H, DH), w_k.reshape(H, DH, C)
    ).reshape(C, H * T)

    consts = dict(
        akT=np.ascontiguousarray(A.astype(bf)),
        wvT=np.ascontiguousarray(w_v.T.astype(bf)),
        woT=np.ascontiguousarray(w_o.T.astype(bf)),
        w1gT=np.ascontiguousarray((w1 * g_ff[None, :]).T.astype(bf)),
        b1e=(b1 + w1 @ b_ff).reshape(1, 2 * C).astype(bf),
        w2T=np.ascontiguousarray(w2.T.astype(bf)),
        b2e=b2.reshape(1, C).astype(bf),
        qb=np.ascontiguousarray(query[0] + (b_o + w_o @ b_v_in)[None, :]).astype(
            np.float32
        ),
        identb=np.eye(128, dtype=bf),
    )
    in_maps = []
    for c in range(NCORES):
        m = dict(consts)
        m["featp"] = featp[c]
        m["featT"] = featT[c].reshape(128, S * NT * 4 * 128)
        m["biasm"] = biasd[c]
        in_maps.append(m)
    return in_maps, P, S, B


def kernel(**inputs):
    from concourse.bass_utils import run_bass_kernel_spmd

    in_maps, P, S, B = _host_prep(inputs)
    chunk = _pick_chunk(P // 128)
    key = (P, S, chunk)
    if key not in _CACHE:
        _CACHE[key] = _build(P, S, chunk=chunk)
    nc = _CACHE[key]
    res = run_bass_kernel_spmd(nc, in_maps, core_ids=list(range(NCORES)))
    out = np.empty((B, T, C), dtype=np.float32)
    for c in range(NCORES):
        o = res.results[c]["outp"]
        for s in range(S):
            out[c * S + s] = o[s * T : (s + 1) * T]
    return out


# revision 37
# speedup vs baseline: 1.0261x; 1.0261x over previous
"""Trainium2 Bass kernel for nn_ConditionPooler (ragged cross-attention pooler).

Algorithm (per core, data-parallel over B=16 scenes, 2 scenes/core on 8 cores):
  scores[n,(h,t)] = feat @ A^T      where A[(h,t),c] = sum_d qh[t,h,d] w_k[h*DH+d,c]
  P = exp(scores/16 + bias)  (A pre-scaled x16 for fp8 range; bias=-30 on padded
                              rows so they vanish from the softmax; b_k_in cancels)
  U[(h,t),c]  = sum_n P[n,(h,t)] feat[n,c]   (PSUM-accumulated across tiles)
  den[1,(h,t)] = sum_n P[n,(h,t)]            (single-row matmul vs ones, PSUM-acc)
  Uhat = U / den; attn_h = Uhat_h @ w_v_h^T; out = attn @ w_o^T + b_o (+ w_o b_v_in)
  h = out + query; z = (h-mu)/std; ff = gelu(z @ (w1*g)^T + b1eff) @ w2^T + b2
  result = h + ff

The scores matmul takes F^T directly from DRAM (host pre-transposes into a
[p, scene, tile, c-chunk, n] layout, so no on-chip transposes or PSUM->SBUF
copies are needed). Everything stays bf16: attention outputs are means over
~2k points, so any per-point weight noise (e.g. fp8 scores ~5%) lands
directly as relative output error — measured 3.4e-2 with fp8e4 DoubleRow
scores vs 5e-3 with bf16. Scenes are padded to a common length P (multiple
of 128); segment boundaries are computed on the host from batch_idx.
"""

import numpy as np

C = 512
T = 32
H = 8
DH = C // H
NCORES = 8

_CACHE = {}


def _apply_tile_patch():
    """This walrus build allows only one sem wait on CTRL-encoded (Drain)
    instructions; TileContext's tail drain carries the whole global clock.
    Split the extra waits onto standalone sync-engine nops."""
    import concourse.tile as tile_mod
    import concourse.mybir as mybir
    from concourse.vector_clock import ScopedClock

    if getattr(tile_mod.TileContext, "_drain_patched", False):
        return

    def _patched(self, tick_clock, wait_clock):
        nc = self.nc
        drain_inst = nc.sync.drain()
        wait_clock.add_sem_waits(
            drain_inst.ins, ScopedClock({None: tick_clock.global_clock})
        )
        si = drain_inst.ins.sync_info
        if si is not None and si.on_wait is not None and len(si.on_wait) > 1:
            waits = list(si.on_wait)
            si.on_wait = waits[:1]
            for w in waits[1:]:
                nop = nc.sync.nop(nofuse=True)
                nsi = nop.ins.sync_info
                if nsi is None:
                    nop.ins.sync_info = mybir.SyncInfo(on_wait=[w], on_update=[])
                else:
                    nsi.on_wait = [w]
        nc.all_engine_barrier()
        assert self.sems is not None
        popped = nc._tile_sem_poison_stack.pop()
        assert popped is self._sem_poison
        nc.clear_and_free_semaphores(list(self.sems.allocated().values()))
        nc.all_engine_barrier()

    tile_mod.TileContext._drain_and_barrier = _patched
    tile_mod.TileContext._drain_patched = True


def _split_multi_waits(nc):
    """This walrus build caps sync waits at 1 per instruction (2 for
    EventSemaphore). Tile emits several on some instructions; hoist the
    extras onto same-engine NoOps inserted just before."""
    import concourse.mybir as mybir

    cnt = [0]
    for f in nc.m.functions:
        for b in f.blocks:
            newlist = []
            for inst in b.instructions:
                si = inst.sync_info
                if si is not None and si.on_wait is not None and len(si.on_wait) > 1:
                    waits = list(si.on_wait)
                    for w in waits[:-1]:
                        cnt[0] += 1
                        nop = mybir.InstNoOp(
                            name=f"I-wsplit-{cnt[0]}", ins=[], outs=[]
                        )
                        nop.engine = inst.engine
                        nop.sync_info = mybir.SyncInfo(on_wait=[w], on_update=[])
                        newlist.append(nop)
                    si.on_wait = waits[-1:]
                newlist.append(inst)
            b.instructions = newlist


def _build(P, S, chunk=6, split=True):
    """Build the per-core SPMD Bass program. P = padded scene length
    (multiple of 128), S = scenes per core."""
    import concourse.bass as bass
    import concourse.mybir as mybir
    import concourse.tile as tile

    _apply_tile_patch()

    f32 = mybir.dt.float32
    bf16 = mybir.dt.bfloat16
    HT = H * T  # 256
    NT = P // 128
    assert P % 128 == 0
    AF = mybir.ActivationFunctionType

    nc = bass.Bass()
    featp = nc.dram_tensor("featp", [S * P, C], bf16, kind="ExternalInput")
    # F^T bf16: [p, s*NT + tile, j(4), n(128)], c = j*128 + p.
    # Per partition one contiguous run per scene.
    featT = nc.dram_tensor("featT", [128, S * NT * 4 * 128], bf16,
                           kind="ExternalInput")
    # A bf16: [p(128), j(4), ht(256)], c = j*128 + p
    a_d = nc.dram_tensor("akT", [C, HT], bf16, kind="ExternalInput")
    biasm_d = nc.dram_tensor("biasm", [128, S * NT], f32, kind="ExternalInput")
    wvT_d = nc.dram_tensor("wvT", [C, C], bf16, kind="ExternalInput")
    woT_d = nc.dram_tensor("woT", [C, C], bf16, kind="ExternalInput")
    w1gT_d = nc.dram_tensor("w1gT", [C, 2 * C], bf16, kind="ExternalInput")
    b1e_d = nc.dram_tensor("b1e", [1, 2 * C], bf16, kind="ExternalInput")
    w2T_d = nc.dram_tensor("w2T", [2 * C, C], bf16, kind="ExternalInput")
    b2e_d = nc.dram_tensor("b2e", [1, C], bf16, kind="ExternalInput")
    qb_d = nc.dram_tensor("qb", [T, C], f32, kind="ExternalInput")
    idb_d = nc.dram_tensor("identb", [128, 128], bf16, kind="ExternalInput")
    outp = nc.dram_tensor("outp", [S * T, C], f32, kind="ExternalOutput")

    NCH = NT // chunk  # chunks per scene
    assert NT % chunk == 0

    with tile.TileContext(nc) as tc:
        with tc.tile_pool(name="const", bufs=1) as const:
            # consts ride the ACT HWDGE ring so they don't delay the feat
            # stream on the sync ring at startup
            akT = const.tile([128, 4, HT], bf16, tag="akT")
            nc.scalar.dma_start(akT[:], a_d.rearrange("(j p) f -> p j f", p=128))
            biasm = const.tile([128, S * NT], f32, tag="biasm")
            nc.scalar.dma_start(biasm[:], biasm_d[:])
            identb = const.tile([128, 128], bf16, tag="identb")
            nc.scalar.dma_start(identb[:], idb_d[:])
            ones1 = const.tile([1, 2 * T], bf16, tag="ones1")
            nc.vector.memset(ones1[:], 1.0)
            onesc = const.tile([128, 1], bf16, tag="onesc")
            nc.vector.memset(onesc[:], 1.0)
            ones1f = const.tile([1, 1], f32, tag="ones1f")
            nc.vector.memset(ones1f[:], 1.0)

            wvT = const.tile([128, 4, C], bf16, tag="wvT")
            woT = const.tile([128, 4, C], bf16, tag="woT")
            w1gT = const.tile([128, 4, 2 * C], bf16, tag="w1gT")
            w2T = const.tile([128, 8, C], bf16, tag="w2T")
            b1e = const.tile([1, 2 * C], bf16, tag="b1e")
            b2e = const.tile([1, C], bf16, tag="b2e")
            qb2 = const.tile([2 * T, C], f32, tag="qb2")

            def load_epi_weights():
                # SWDGE so these 3.2MB don't head-of-line block the feat
                # stream on HWDGE; issued mid-stream, needed only at the end.
                g = nc.gpsimd
                g.dma_start(wvT[:], wvT_d.rearrange("(j p) f -> p j f", p=128))
                g.dma_start(woT[:], woT_d.rearrange("(j p) f -> p j f", p=128))
                g.dma_start(w1gT[:], w1gT_d.rearrange("(j p) f -> p j f", p=128))
                g.dma_start(w2T[:], w2T_d.rearrange("(j p) f -> p j f", p=128))
                g.dma_start(b1e[:], b1e_d[:])
                g.dma_start(b2e[:], b2e_d[:])
                for s in range(S):
                    g.dma_start(qb2[s * T : (s + 1) * T, :], qb_d[:])

            featv = featp.rearrange(
                "(s g i p) c -> s g p i c", p=128, i=chunk, g=NCH
            )
            # F^T bf16 stream: per (scene, chunk) [128, chunk, 4, 128]
            ftv = featT.rearrange(
                "p (s g i j n) -> s g p i j n", s=S, g=NCH, i=chunk, j=4, n=128
            )

            from collections import deque

            with (
                tc.tile_pool(name="epiA", bufs=1) as epiA,
                tc.tile_pool(name="ftp", bufs=2) as ftp,
                tc.tile_pool(name="fb", bufs=3) as fpool,
                tc.tile_pool(name="sb", bufs=4) as spool,
                tc.tile_pool(name="rd", bufs=2) as rdp,
                tc.tile_pool(name="epi", bufs=1) as epi,
                tc.tile_pool(name="psU", bufs=1, space="PSUM") as psU_pool,
                tc.tile_pool(name="pssc", bufs=2, space="PSUM") as pssc,
                tc.tile_pool(name="psd", bufs=2, space="PSUM") as psd,
                tc.tile_pool(name="pst", bufs=1, space="PSUM") as pst,
                tc.tile_pool(name="psac", bufs=1, space="PSUM") as psac,
            ):
                Uhat = [
                    epiA.tile([128, 2, C], bf16, tag=f"Uh{s}", name=f"Uh{s}")
                    for s in range(S)
                ]
                rdT = epiA.tile([128, 2, S], f32, tag="rdT")
                epsc = epiA.tile([T, 1], f32, tag="epsc")
                nc.vector.memset(epsc[:], 1e-5)

                def emit_U(st):
                    PT, F, sp, ip, Upsp, denp = st
                    for h2 in range(2):
                        nc.tensor.matmul(
                            Upsp[h2][:],
                            PT[:, h2 * 128 : (h2 + 1) * 128],
                            F,
                            start=(ip == 0),
                            stop=(ip == NT - 1),
                        )
                    nc.tensor.matmul(
                        denp[:1, :HT],
                        onesc[:],
                        PT[:],
                        start=(ip == 0),
                        stop=(ip == NT - 1),
                    )

                def finish_scene(st):
                    emit_U(st)
                    sp, Upsp, denp = st[2], st[4], st[5]
                    # rden, transpose rden into [128, 2] (cols 256:258 of
                    # the same den bank), scale U -> Uhat
                    rden = rdp.tile([1, HT], f32, tag="rden")
                    nc.vector.reciprocal(rden[:], denp[:1, :HT])
                    for h2 in range(2):
                        nc.tensor.matmul(
                            denp[:, HT + h2 : HT + h2 + 1],
                            rden[:, h2 * 128 : (h2 + 1) * 128],
                            ones1f[:],
                            start=True,
                            stop=True,
                        )
                    nc.vector.tensor_copy(rdT[:, :, sp], denp[:, HT : HT + 2])
                    # split across ACT and DVE so the halves run in parallel
                    nc.scalar.activation(
                        Uhat[sp][:, 0, :],
                        Upsp[0][:],
                        AF.Copy,
                        scale=rdT[:, 0, sp : sp + 1],
                    )
                    nc.vector.tensor_scalar_mul(
                        Uhat[sp][:, 1, :], Upsp[1][:], rdT[:, 1, sp : sp + 1]
                    )

                def scene_epilogue(sp):
                    """Per-scene tail, as closures spread across the next
                    scene's tile stream (or drained back-to-back at the end)."""
                    UT = epi.tile([128, 4, HT], bf16, tag="UT", name=f"UT{sp}")
                    at_sb = epi.tile(
                        [128, 4, T], bf16, tag="at_sb", name=f"ats{sp}"
                    )
                    h_sb = epi.tile([T, C], f32, tag="h", name=f"h{sp}")
                    mu = epi.tile([T, 1], f32, tag="mu", name=f"mu{sp}")
                    cen = epi.tile([T, C], f32, tag="cen", name=f"cen{sp}")
                    ssq = epi.tile([T, 1], f32, tag="ssq", name=f"ssq{sp}")
                    rstd = epi.tile([T, 1], f32, tag="rstd", name=f"rstd{sp}")
                    z = epi.tile([T, C], bf16, tag="z", name=f"z{sp}")
                    zT = epi.tile([128, 4, T], bf16, tag="zT", name=f"zT{sp}")
                    gm = epi.tile([T, 2, C], bf16, tag="gm", name=f"gm{sp}")
                    gmT = epi.tile([128, 8, T], bf16, tag="gmT", name=f"gmT{sp}")
                    fin = epi.tile([T, C], f32, tag="fin", name=f"fin{sp}")

                    def p_ut():
                        ps_u = pst.tile(
                            [128, 4, HT], bf16, tag="tre", name=f"psu{sp}"
                        )
                        for h2 in range(2):
                            for jc in range(4):
                                nc.tensor.transpose(
                                    ps_u[:, jc, h2 * 128 : (h2 + 1) * 128],
                                    Uhat[sp][:, h2, jc * 128 : (jc + 1) * 128],
                                    identb[:],
                                )
                        nc.vector.tensor_copy(UT[:], ps_u[:])

                    def p_at():
                        # [128, 4, 128] so the per-partition bytes match the
                        # "acc" tag (shares one PSUM bank with ph/pf/po)
                        at_ps = psac.tile(
                            [128, 4, 128], f32, tag="acc", name=f"at{sp}"
                        )
                        for gq in range(4):
                            for hh in range(2):
                                h = 2 * gq + hh
                                for jc in range(4):
                                    nc.tensor.matmul(
                                        at_ps[hh * 64 : (hh + 1) * 64, gq, :T],
                                        wvT[:, jc, h * DH : (h + 1) * DH],
                                        UT[:, jc, h * T : (h + 1) * T],
                                        start=(jc == 0),
                                        stop=(jc == 3),
                                    )
                        nc.any.tensor_copy(at_sb[:], at_ps[:, :, :T])

                    def p_ph():
                        ph = psac.tile([128, C], f32, tag="acc", name=f"ph{sp}")
                        for gq in range(4):
                            nc.tensor.matmul(
                                ph[:T, :],
                                at_sb[:, gq, :],
                                woT[:, gq, :],
                                start=(gq == 0),
                                stop=(gq == 3),
                            )
                        nc.vector.tensor_add(
                            h_sb[:], ph[:T, :], qb2[sp * T : (sp + 1) * T, :]
                        )

                    def p_ln():
                        ssum = epi.tile([T, 1], f32, tag="ssum", name=f"ssum{sp}")
                        nc.vector.reduce_sum(
                            ssum[:], h_sb[:], axis=mybir.AxisListType.X
                        )
                        nc.scalar.mul(mu[:], ssum[:], 1.0 / C)
                        nc.vector.tensor_scalar_sub(cen[:], h_sb[:], mu[:])
                        sq = epi.tile([T, C], f32, tag="sq", name=f"sq{sp}")
                        nc.scalar.activation(
                            sq[:], cen[:], AF.Square, accum_out=ssq[:]
                        )
                        std = epi.tile([T, 1], f32, tag="std", name=f"std{sp}")
                        nc.scalar.activation(
                            std[:], ssq[:], AF.Sqrt, bias=epsc[:], scale=1.0 / C
                        )
                        nc.vector.reciprocal(rstd[:], std[:])
                        nc.vector.tensor_scalar_mul(z[:], cen[:], rstd[:])

                    def p_zt():
                        ps_z = pst.tile(
                            [128, 4, HT], bf16, tag="tre", name=f"psz{sp}"
                        )
                        for jc in range(4):
                            nc.tensor.transpose(
                                ps_z[:, jc, :T],
                                z[:, jc * 128 : (jc + 1) * 128],
                                identb[:T, :T],
                            )
                        nc.any.tensor_copy(zT[:], ps_z[:, :, :T])

                    def p_ff1(half):
                        def run():
                            pf = psac.tile(
                                [128, C], f32, tag="acc", name=f"pf{sp}{half}"
                            )
                            for jc in range(4):
                                nc.tensor.matmul(
                                    pf[:T, :],
                                    zT[:, jc, :],
                                    w1gT[:, jc, half * C : (half + 1) * C],
                                    start=(jc == 0),
                                    stop=False,
                                )
                            nc.tensor.matmul(
                                pf[:T, :],
                                ones1[:, :T],
                                b1e[:, half * C : (half + 1) * C],
                                start=False,
                                stop=True,
                            )
                            nc.scalar.activation(gm[:, half, :], pf[:T, :], AF.Gelu)

                        return run

                    def p_gmt():
                        ps_g = pst.tile(
                            [128, 4, HT], bf16, tag="tre", name=f"psg{sp}"
                        )
                        for half in range(2):
                            for jc in range(4):
                                nc.tensor.transpose(
                                    ps_g[:, jc, half * T : (half + 1) * T],
                                    gm[:, half, jc * 128 : (jc + 1) * 128],
                                    identb[:T, :T],
                                )
                        for half in range(2):
                            nc.vector.tensor_copy(
                                gmT[:, half * 4 : (half + 1) * 4, :],
                                ps_g[:, :, half * T : (half + 1) * T],
                            )

                    def p_ff2():
                        po = psac.tile([128, C], f32, tag="acc", name=f"po{sp}")
                        for k in range(8):
                            nc.tensor.matmul(
                                po[:T, :],
                                gmT[:, k, :],
                                w2T[:, k, :],
                                start=(k == 0),
                                stop=False,
                            )
                        nc.tensor.matmul(
                            po[:T, :], ones1[:, :T], b2e[:], start=False, stop=True
                        )
                        nc.vector.tensor_add(fin[:], h_sb[:], po[:T, :])
                        nc.sync.dma_start(
                            outp[sp * T : (sp + 1) * T, :], fin[:]
                        )

                    return [
                        p_ut,
                        p_at,
                        p_ph,
                        p_ln,
                        p_zt,
                        p_ff1(0),
                        p_ff1(1),
                        p_gmt,
                        p_ff2,
                    ]

                epi_q = deque()

                def load_small_weights():
                    # SWDGE: fires immediately; only the tiny tensors (~70KB)
                    g = nc.gpsimd
                    g.dma_start(b1e[:], b1e_d[:])
                    g.dma_start(b2e[:], b2e_d[:])
                    for s2 in range(S):
                        g.dma_start(qb2[s2 * T : (s2 + 1) * T, :], qb_d[:])

                # Epilogue weights (3.2MB) ride the sync HWDGE queue in
                # pieces so their FIFO position (not issue time) paces them
                # between feat chunks.
                wvv = wvT_d.rearrange("(j p) f -> p j f", p=128)
                wov = woT_d.rearrange("(j p) f -> p j f", p=128)
                w1v = w1gT_d.rearrange("(j p) f -> p j f", p=128)
                w2v = w2T_d.rearrange("(j p) f -> p j f", p=128)
                weight_sched = {
                    (0, min(1, NCH - 1)): [
                        lambda: nc.sync.dma_start(wvT[:], wvv),
                    ],
                    (0, min(2, NCH - 1)): [
                        lambda: nc.sync.dma_start(woT[:], wov),
                    ],
                    (min(1, S - 1), 0): [
                        lambda: nc.sync.dma_start(w1gT[:, :, :C], w1v[:, :, :C]),
                        lambda: nc.sync.dma_start(w1gT[:, :, C:], w1v[:, :, C:]),
                    ],
                    (min(1, S - 1), min(1, NCH - 1)): [
                        lambda: nc.sync.dma_start(w2T[:, :4, :], w2v[:, :4, :]),
                    ],
                    (min(1, S - 1), min(2, NCH - 1)): [
                        lambda: nc.sync.dma_start(w2T[:, 4:, :], w2v[:, 4:, :]),
                    ],
                }

                pend = None
                for s in range(S):
                    Ups = [
                        psU_pool.tile(
                            [128, C], f32, tag=f"U{h2}", name=f"U{s}{h2}"
                        )
                        for h2 in range(2)
                    ]
                    den_ps = psd.tile([128, HT + 2], f32, tag="den", name=f"d{s}")
                    for g in range(NCH):
                        FT = ftp.tile([128, chunk, 4, 128], bf16, tag="FT")
                        Fc = fpool.tile([128, chunk, C], bf16, tag="F")
                        if s == 0 and g == 0:
                            # 2-piece split so the first tiles land early
                            # (HWDGE has ~0.6us flat cost per DMA — don't
                            # over-fragment)
                            cut = min(2, chunk)
                            nc.sync.dma_start(FT[:, :cut], ftv[s, g][:, :cut])
                            nc.sync.dma_start(Fc[:, :cut], featv[s, g][:, :cut])
                            nc.sync.dma_start(FT[:, cut:], ftv[s, g][:, cut:])
                            nc.sync.dma_start(Fc[:, cut:], featv[s, g][:, cut:])
                            load_small_weights()
                        else:
                            nc.sync.dma_start(FT[:], ftv[s, g])
                            nc.sync.dma_start(Fc[:], featv[s, g])
                        for w_fn in weight_sched.get((s, g), ()):
                            w_fn()
                        for ii in range(chunk):
                            i = g * chunk + ii
                            F = Fc[:, ii, :]
                            ps_s = pssc.tile([128, HT], f32, tag="sc")
                            for j in range(4):
                                nc.tensor.matmul(
                                    ps_s[:],
                                    FT[:, ii, j, :],
                                    akT[:, j, :],
                                    start=(j == 0),
                                    stop=(j == 3),
                                )
                            PT = spool.tile([128, HT], bf16, tag="PT")
                            nc.scalar.activation(
                                PT[:],
                                ps_s[:],
                                AF.Exp,
                                bias=biasm[:, s * NT + i : s * NT + i + 1],
                            )
                            if pend is not None:
                                if pend[2] != s:
                                    finish_scene(pend)
                                    epi_q.extend(scene_epilogue(pend[2]))
                                else:
                                    emit_U(pend)
                            if epi_q and i % 2 == 0:
                                # pop every 2nd tile: keeps the overlapped
                                # epilogue behind its weight deliveries
                                epi_q.popleft()()
                            pend = (PT, F, s, i, Ups, den_ps)
                finish_scene(pend)
                epi_q.extend(scene_epilogue(pend[2]))
                while epi_q:
                    epi_q.popleft()()

    if split:
        _split_multi_waits(nc)
    return nc


def _pick_chunk(NT):
    for c in (6, 5, 4, 3, 2, 1):
        if NT % c == 0:
            return c
    return 1


def _host_prep(inputs):
    import ml_dtypes

    bf = ml_dtypes.bfloat16

    feat = np.asarray(inputs["feat"], dtype=np.float32)
    batch_idx = np.asarray(inputs["batch_idx"]).astype(np.int64)
    B = int(np.asarray(inputs["batch_size"]))
    query = np.asarray(inputs["query"], dtype=np.float32)
    g_q = np.asarray(inputs["g_q"], np.float32)
    b_q = np.asarray(inputs["b_q"], np.float32)
    w_q = np.asarray(inputs["w_q"], np.float32)
    w_k = np.asarray(inputs["w_k"], np.float32)
    w_v = np.asarray(inputs["w_v"], np.float32)
    b_q_in = np.asarray(inputs["b_q_in"], np.float32)
    b_v_in = np.asarray(inputs["b_v_in"], np.float32)
    w_o = np.asarray(inputs["w_o"], np.float32)
    b_o = np.asarray(inputs["b_o"], np.float32)
    g_ff = np.asarray(inputs["g_ff"], np.float32)
    b_ff = np.asarray(inputs["b_ff"], np.float32)
    w1 = np.asarray(inputs["w1"], np.float32)
    b1 = np.asarray(inputs["b1"], np.float32)
    w2 = np.asarray(inputs["w2"], np.float32)
    b2 = np.asarray(inputs["b2"], np.float32)

    S = B // NCORES
    counts = np.bincount(batch_idx, minlength=B)
    offs = np.concatenate([[0], np.cumsum(counts)])
    NT = max(1, int(np.ceil(counts.max() / 128)))
    while _pick_chunk(NT) < 3 and NT > 2:
        NT += 1
    P = NT * 128

    featp = np.zeros((NCORES, S * P, C), dtype=bf)
    biasm = np.full((NCORES, S, NT * 128), -30.0, dtype=np.float32)
    for b in range(B):
        c, s = divmod(b, S)
        n = counts[b]
        featp[c, s * P : s * P + n] = feat[offs[b] : offs[b + 1]].astype(bf)
        biasm[c, s, :n] = 0.0
    # bias laid out [128, S*NT] (partition = point-within-tile)
    biasd = (
        biasm.reshape(NCORES, S, NT, 128)
        .transpose(0, 3, 1, 2)
        .reshape(NCORES, 128, S * NT)
    )
    # F^T bf16: [core, p, s, tile, j, n] with c = j*128 + p
    f4 = featp.reshape(NCORES, S, NT, 128, 4, 128)
    # dims: [core, s, tile, n, j, p] -> transpose to [core, p, s, tile, j, n]
    featT = np.ascontiguousarray(f4.transpose(0, 5, 1, 2, 4, 3))

    # query-side fold (host; tiny)
    q = query[0]
    mu = q.mean(-1, keepdims=True)
    var = ((q - mu) ** 2).mean(-1, keepdims=True)
    qn = (q - mu) / np.sqrt(var + 1e-5) * g_q + b_q
    qh = (qn @ w_q.T + b_q_in) / np.sqrt(DH)  # [T, C]
    A = np.einsum(
        "thd,hdc->cht", qh.reshape(T, H, DH), w_k.reshape(H, DH, C)
    ).reshape(C, H * T)

    consts = dict(
        akT=np.ascontiguousarray(A.astype(bf)),
        wvT=np.ascontiguousarray(w_v.T.astype(bf)),
        woT=np.ascontiguousarray(w_o.T.astype(bf)),
        w1gT=np.ascontiguousarray((w1 * g_ff[None, :]).T.astype(bf)),
        b1e=(b1 + w1 @ b_ff).reshape(1, 2 * C).astype(bf),
        w2T=np.ascontiguousarray(w2.T.astype(bf)),
        b2e=b2.reshape(1, C).astype(bf),
        qb=np.ascontiguousarray(query[0] + (b_o + w_o @ b_v_in)[None, :]).astype(
            np.float32
        ),
        identb=np.eye(128, dtype=bf),
    )
    in_maps = []
    for c in range(NCORES):
        m = dict(consts)
        m["featp"] = featp[c]
        m["featT"] = featT[c].reshape(128, S * NT * 4 * 128)
        m["biasm"] = biasd[c]
        in_maps.append(m)
    return in_maps, P, S, B


def kernel(**inputs):
    from concourse.bass_utils import run_bass_kernel_spmd

    in_maps, P, S, B = _host_prep(inputs)
    chunk = _pick_chunk(P // 128)
    key = (P, S, chunk)
    if key not in _CACHE:
        _CACHE[key] = _build(P, S, chunk=chunk)
    nc = _CACHE[key]
    res = run_bass_kernel_spmd(nc, in_maps, core_ids=list(range(NCORES)))
    out = np.empty((B, T, C), dtype=np.float32)
    for c in range(NCORES):
        o = res.results[c]["outp"]
        for s in range(S):
            out[c * S + s] = o[s * T : (s + 1) * T]
    return out
